# revision 23
# baseline (speedup 1.0000x reference)
"""ASAScorer GNN kernel for 8 Trainium2 NeuronCores.

Strategy (col-sharded, fully edge-dense per core):
  - Host: sort nodes by in-degree, deal round-robin to 8 cores (6250 nodes
    each), pack each core's nodes into a 128x49 (partition x tile) grid.
    For each tile, pad every node's neighbor list to shared per-tile slot
    counts; build int16 dma_gather index lists (two table ranges, since
    int16 tops out at 32767 rows) plus a 0/1 validity mask.
  - Launch 1 (per core): for each tile, dma_gather the neighbor rows of
    x_ext [50002, 64] into SBUF ([128 nodes, K slots, 64]), then dense
    per-tile compute: segment-max (xq), attention scores u[col]+v[row]
    via folded weights, leaky-relu, masked segment softmax, weighted
    segment-sum (xp), and per-node scalars a1/b2/le3.xp.
  - Host: concatenate the 8 a1 strips and expand a1[row] per slot.
  - Launch 2 (per core): masked segment-sum of a1 slots -> agg,
    fit = sigmoid(agg - indeg*b2 + le3.xp + b3), out = xp * fit.

Table split: x_ext2 = [pad | x | pad], pad = -1e30 rows.
  group LO: x rows [0, 32767)   -> table rows 1+r      (sentinel 0)
  group HI: x rows [32767, N)   -> table rows 1+r      (base 32768,
            local idx r-32767 in [0,17233), sentinel local 17233)
"""

import numpy as np

import concourse.bass as bass
import concourse.bacc as bacc
import concourse.tile as tile
from concourse import mybir
from concourse.bass_utils import run_bass_kernel_spmd

N = 50000
D = 64
NCORES = 8
P = 128
STRIP = N // NCORES          # 6250
T = (STRIP + P - 1) // P     # 49 tiles
CAP = P * T                  # 6272 grid cells per core
SPLIT = 32767                # x rows < SPLIT -> LO group
VEXT = N + 2                 # x_ext2 rows
NEG_SLOPE = 0.2
F32 = mybir.dt.float32
I16 = mybir.dt.int16
F32SZ = 4

_compiled = {}
TRACE = False
LAST_TIMES = []


# ----------------------------------------------------------------------------
# Host-side index preparation (pure index/permutation work on edge_index)
# ----------------------------------------------------------------------------

def _wrap16(lists):
    """Pack gather positions into the dma_gather int16 SBUF layout.

    lists: int32 array [n] of table indices for positions 0..n-1 (n % 128
    == 0).  Returns [128, n // 16] int16: element q sits at partition
    q % 16, col q // 16, replicated across the eight 16-partition groups.
    """
    n = lists.shape[0]
    a = lists.reshape(n // 16, 16).astype(np.int16).T    # [16, n//16]
    return np.tile(a, (8, 1))                            # [128, n//16]


def _prep(edge_index):
    row = np.asarray(edge_index[0], dtype=np.int64)
    col = np.asarray(edge_index[1], dtype=np.int64)
    E = row.shape[0]

    deg = np.bincount(col, minlength=N)
    order = np.argsort(col, kind="stable")
    rows_sorted = row[order]
    starts = np.zeros(N + 1, dtype=np.int64)
    np.cumsum(deg, out=starts[1:])

    # per-node neighbor lists, LO rows first, then HI rows
    Kmax = int(deg.max())
    within = np.arange(E, dtype=np.int64) - starts[col[order]]
    key = rows_sorted >= SPLIT
    sub_order = np.lexsort((key,))  # stable by (edge order), grouped later
    # reorder each node's list so LO rows come first: sort edges by (col, hi?)
    order2 = np.lexsort((rows_sorted >= SPLIT, col[order]))
    rows2 = rows_sorted[order2]
    col2 = col[order][order2]
    within2 = np.arange(E, dtype=np.int64) - starts[col2]
    nbr = np.full((N, Kmax), -1, dtype=np.int64)
    nbr[col2, within2] = rows2
    deg_lo = np.bincount(col, weights=(row < SPLIT), minlength=N).astype(np.int64)
    deg_hi = deg - deg_lo

    # node -> (core, position i); i -> tile t = i//128, partition p = i%128
    # sort by (deg_lo, deg_hi) so tiles are homogeneous in both slot groups
    rank = np.lexsort((-deg_hi, -deg_lo))
    perm = np.empty((NCORES, STRIP), dtype=np.int64)
    perm[np.arange(N) % NCORES, np.arange(N) // NCORES] = rank

    # per-tile slot counts (shared across cores)
    K_lo = np.zeros(T, dtype=np.int64)
    K_hi = np.zeros(T, dtype=np.int64)
    for t in range(T):
        lo, hi = t * P, min((t + 1) * P, STRIP)
        K_lo[t] = int(deg_lo[perm[:, lo:hi]].max())
        K_hi[t] = int(deg_hi[perm[:, lo:hi]].max())
        if K_lo[t] + K_hi[t] == 0:
            K_lo[t] = 1
    K_t = (K_lo + K_hi).astype(np.int64)
    offs = np.zeros(T + 1, dtype=np.int64)
    np.cumsum(K_t, out=offs[1:])
    SF = int(offs[-1])

    # mask [NCORES, P, SF]; gather index lists in slot-major position order
    mask = np.zeros((NCORES, P, SF), dtype=np.float32)
    rowof = np.full((NCORES, P, SF), -1, dtype=np.int64)   # x row per slot
    ilo_cols = 8 * int(K_lo.sum())
    ihi_cols = 8 * int(K_hi.sum())
    idx_lo = np.empty((NCORES, P, ilo_cols), dtype=np.int16)
    idx_hi = np.empty((NCORES, P, ihi_cols), dtype=np.int16)
    olo = ohi = 0
    lo_offs = np.zeros(T + 1, dtype=np.int64)
    hi_offs = np.zeros(T + 1, dtype=np.int64)
    for t in range(T):
        kl, kh = int(K_lo[t]), int(K_hi[t])
        lo, hi = t * P, min((t + 1) * P, STRIP)
        np_t = hi - lo
        nodes = perm[:, lo:hi]                      # [NCORES, np_t]
        dl = deg_lo[nodes]
        dh = deg_hi[nodes]
        dg = deg[nodes]
        blk = nbr[nodes]                            # [NCORES, np_t, Kmax]
        # slot k in [0, kl): k-th LO row; slot kl+k: k-th HI row
        ar_l = np.arange(kl)
        ar_h = np.arange(kh)
        # LO table indices (sentinel 0); position q = k*128 + p
        lo_idx = np.zeros((NCORES, P, kl), dtype=np.int64)
        valid_l = ar_l[None, None, :] < dl[:, :, None]
        if kl:
            take = np.minimum(ar_l[None, None, :], (dl - 1)[:, :, None].clip(0))
            vals = np.take_along_axis(blk, take, axis=2)[:, :, :kl]
            lo_idx[:, :np_t, :] = np.where(valid_l, vals + 1, 0)
            mask[:, :np_t, offs[t]:offs[t] + kl] = valid_l
        # HI table indices (base 32768 => local = row - SPLIT; sentinel 17233)
        hi_idx = np.full((NCORES, P, kh), VEXT - 1 - 32768, dtype=np.int64)
        if kh:
            take = np.minimum(dl[:, :, None] + ar_h[None, None, :],
                              (dg - 1)[:, :, None].clip(0))
            vals = np.take_along_axis(blk, take.clip(0, Kmax - 1), axis=2)[:, :, :kh]
            valid_h = ar_h[None, None, :] < dh[:, :, None]
            hi_idx[:, :np_t, :] = np.where(valid_h, vals - SPLIT, VEXT - 1 - 32768)
            mask[:, :np_t, offs[t] + kl:offs[t] + kl + kh] = valid_h
        # record x rows for the a1 expansion
        if kl:
            r = np.where(valid_l, lo_idx[:, :np_t] - 1, -1)
            rowof[:, :np_t, offs[t]:offs[t] + kl] = r
        if kh:
            r = np.where(valid_h, hi_idx[:, :np_t] + SPLIT, -1)
            rowof[:, :np_t, offs[t] + kl:offs[t] + kl + kh] = r
        # wrap into dma_gather layout, position q = k*128 + p
        for c in range(NCORES):
            if kl:
                flat = lo_idx[c].T.reshape(-1)      # [kl*128] q=k*128+p
                idx_lo[c, :, olo:olo + 8 * kl] = _wrap16(flat)
            if kh:
                flat = hi_idx[c].T.reshape(-1)
                idx_hi[c, :, ohi:ohi + 8 * kh] = _wrap16(flat)
        lo_offs[t + 1] = lo_offs[t] + 8 * kl
        hi_offs[t + 1] = hi_offs[t] + 8 * kh
        olo += 8 * kl
        ohi += 8 * kh

    # a1 position per node: a1_full layout is concat of [P, T] strips
    ip = np.arange(STRIP)
    a1pos = np.empty(N + 1, dtype=np.int64)
    a1pos[perm] = (np.arange(NCORES)[:, None] * CAP
                   + (ip % P)[None, :] * T + (ip // P)[None, :])
    a1pos[N] = 0
    idx2 = a1pos[np.where(rowof >= 0, rowof, N)]    # [NCORES, P, SF]

    deg_grid = np.zeros((NCORES, P, T), dtype=np.float32)
    deg_grid[:, ip % P, ip // P] = deg[perm].astype(np.float32)

    return dict(idx_lo=idx_lo, idx_hi=idx_hi, mask=mask, idx2=idx2,
                deg_grid=deg_grid, K_lo=[int(k) for k in K_lo],
                K_hi=[int(k) for k in K_hi], K_t=[int(k) for k in K_t],
                offs=offs, lo_offs=lo_offs, hi_offs=hi_offs, SF=SF, perm=perm)


# ----------------------------------------------------------------------------
# Launch 1: gather + attention softmax + weighted pooling
# ----------------------------------------------------------------------------

def _build_l1(K_lo, K_hi, offs, lo_offs, hi_offs, SF, t_end=T, skip=()):
    from concourse.library_config import mlp
    nc = bacc.Bacc("TRN2", target_bir_lowering=False)
    LO_COLS = int(lo_offs[-1])
    HI_COLS = int(hi_offs[-1])
    xext = nc.dram_tensor("xext", [VEXT, D], F32, kind="ExternalInput")
    idxlo_d = nc.dram_tensor("idxlo", [P, LO_COLS], I16, kind="ExternalInput")
    idxhi_d = nc.dram_tensor("idxhi", [P, max(HI_COLS, 1)], I16,
                             kind="ExternalInput")
    mask_d = nc.dram_tensor("mask", [P, SF], F32, kind="ExternalInput")
    atthi_d = nc.dram_tensor("atthi", [P, D], F32, kind="ExternalInput")
    cvec_d = nc.dram_tensor("cvec", [P, D], F32, kind="ExternalInput")
    u0_d = nc.dram_tensor("u0", [P, 1], F32, kind="ExternalInput")
    le1_d = nc.dram_tensor("le1", [P, D], F32, kind="ExternalInput")
    le2_d = nc.dram_tensor("le2", [P, D], F32, kind="ExternalInput")
    le3_d = nc.dram_tensor("le3", [P, D], F32, kind="ExternalInput")
    le1b_d = nc.dram_tensor("le1b", [P, 1], F32, kind="ExternalInput")

    xp_o = nc.dram_tensor("xp_out", [P, T, D], F32, kind="ExternalOutput")
    a1_o = nc.dram_tensor("a1_out", [P, T], F32, kind="ExternalOutput")
    b2_o = nc.dram_tensor("b2_out", [P, T], F32, kind="ExternalOutput")
    l3_o = nc.dram_tensor("l3_out", [P, T], F32, kind="ExternalOutput")

    nc.gpsimd.load_library(mlp)
    with tile.TileContext(nc) as tc:
        with tc.tile_pool(name="single", bufs=1) as single, \
             tc.tile_pool(name="gpool", bufs=4) as gpool, \
             tc.tile_pool(name="spool", bufs=6) as spool:
            idxlo_sb = single.tile([P, LO_COLS], I16)
            idxhi_sb = single.tile([P, max(HI_COLS, 1)], I16)
            mask_sb = single.tile([P, SF], F32)
            atthi = single.tile([P, D], F32)
            cvec = single.tile([P, D], F32)
            u0 = single.tile([P, 1], F32)
            le1 = single.tile([P, D], F32)
            le2 = single.tile([P, D], F32)
            le3 = single.tile([P, D], F32)
            le1b = single.tile([P, 1], F32)
            xp_all = single.tile([P, T, D], F32)
            den_all = single.tile([P, T], F32)
            rden_all = single.tile([P, T], F32)
            a1_all = single.tile([P, T], F32)
            b2_all = single.tile([P, T], F32)
            l3_all = single.tile([P, T], F32)

            nc.sync.dma_start(idxlo_sb[:], idxlo_d[:])
            nc.sync.dma_start(idxhi_sb[:], idxhi_d[:])
            nc.sync.dma_start(mask_sb[:], mask_d[:])
            nc.sync.dma_start(atthi[:], atthi_d[:])
            nc.sync.dma_start(cvec[:], cvec_d[:])
            nc.sync.dma_start(u0[:], u0_d[:])
            nc.sync.dma_start(le1[:], le1_d[:])
            nc.sync.dma_start(le2[:], le2_d[:])
            nc.sync.dma_start(le3[:], le3_d[:])
            nc.sync.dma_start(le1b[:], le1b_d[:])

            for t in range(t_end):
                kl, kh = K_lo[t], K_hi[t]
                K = kl + kh
                off = int(offs[t])
                g = gpool.tile([P, K, D], F32, tag="g")
                CH = 8  # max slots per dma_gather (1024-descriptor ring limit)
                for c0 in range(0, kl, CH):
                    n = min(CH, kl - c0)
                    o0 = int(lo_offs[t]) + 8 * c0
                    nc.gpsimd.dma_gather(
                        g[:, c0:c0 + n, :], xext[0:32768, :],
                        idxlo_sb[:, o0:o0 + 8 * n], n * P, n * P, D)
                for c0 in range(0, kh, CH):
                    n = min(CH, kh - c0)
                    o0 = int(hi_offs[t]) + 8 * c0
                    nc.gpsimd.dma_gather(
                        g[:, kl + c0:kl + c0 + n, :], xext[32768:VEXT, :],
                        idxhi_sb[:, o0:o0 + 8 * n], n * P, n * P, D)
                # v[p, k] = dot(g[p, k, :], att_hi)
                gv = gpool.tile([P, K, D], F32, tag="gv")
                v = spool.tile([P, K], F32, tag="v")
                if "v" not in skip:
                    nc.vector.tensor_mul(
                        gv[:], g[:],
                        atthi[:].unsqueeze(1).to_broadcast([P, K, D]))
                    nc.vector.tensor_reduce(v[:], gv[:],
                                            axis=mybir.AxisListType.X,
                                            op=mybir.AluOpType.add)
                else:
                    nc.vector.memset(v[:], 0.0)
                # xq[p, :] = max_k g[p, k, :]
                xq = spool.tile([P, D], F32, tag="xq")
                if "xq" not in skip:
                    nc.vector.tensor_reduce(xq[:], g[:].transpose([0, 2, 1]),
                                            axis=mybir.AxisListType.X,
                                            op=mybir.AluOpType.max)
                else:
                    nc.vector.memset(xq[:], 0.0)
                # u[p] = dot(xq, c) + u0
                scr = spool.tile([P, D], F32, tag="scr")
                u = spool.tile([P, 1], F32, tag="u")
                nc.vector.tensor_mul(scr[:], xq[:], cvec[:])
                nc.vector.tensor_reduce(u[:], scr[:], axis=mybir.AxisListType.X,
                                        op=mybir.AluOpType.add)
                nc.vector.tensor_add(u[:], u[:], u0[:])
                # s = leaky_relu(v + u)
                sp = spool.tile([P, K], F32, tag="sp")
                nc.vector.tensor_scalar_add(sp[:], v[:], u[:, 0:1])
                s = spool.tile([P, K], F32, tag="s")
                nc.vector.scalar_tensor_tensor(
                    out=s[:], in0=sp[:], scalar=NEG_SLOPE, in1=sp[:],
                    op0=mybir.AluOpType.mult, op1=mybir.AluOpType.max)
                # t_ = (s + 1000) * mask: pads -> 0, reals ~1000 (shift
                # cancels in softmax; exp(0 - m) underflows to 0 for pads)
                t_ = spool.tile([P, K], F32, tag="t_")
                nc.vector.scalar_tensor_tensor(
                    out=t_[:], in0=s[:], scalar=1000.0,
                    in1=mask_sb[:, off:off + K],
                    op0=mybir.AluOpType.add, op1=mybir.AluOpType.mult)
                mneg = spool.tile([P, 1], F32, tag="mneg")
                nc.vector.tensor_reduce(mneg[:], t_[:],
                                        axis=mybir.AxisListType.X,
                                        op=mybir.AluOpType.max, negate=True)
                e = spool.tile([P, K], F32, tag="e")
                nc.scalar.activation(e[:], t_[:],
                                     mybir.ActivationFunctionType.Exp,
                                     bias=mneg[:, 0:1], scale=1.0,
                                     accum_out=den_all[:, t:t + 1])
                # xp_raw = sum_k e_k * g_k  (scaled by 1/den after the loop)
                wg = gpool.tile([P, K, D], F32, tag="wg")
                if "xp" not in skip:
                    nc.vector.tensor_mul(
                        wg[:], g[:], e[:].unsqueeze(2).to_broadcast([P, K, D]))
                    nc.vector.tensor_reduce(xp_all[:, t, :],
                                            wg[:].transpose([0, 2, 1]),
                                            axis=mybir.AxisListType.X,
                                            op=mybir.AluOpType.add)
                else:
                    nc.vector.memset(xp_all[:, t, :], 0.0)

            nc.vector.reciprocal(rden_all[:], den_all[:])
            nc.vector.tensor_mul(
                xp_all[:], xp_all[:],
                rden_all[:].unsqueeze(2).to_broadcast([P, T, D]))
            # batched per-node scalars over the whole [P, T, D] grid
            scrT = single.tile([P, T, D], F32)
            nc.vector.tensor_mul(scrT[:], xp_all[:],
                                 le1[:].unsqueeze(1).to_broadcast([P, T, D]))
            nc.vector.tensor_reduce(a1_all[:], scrT[:],
                                    axis=mybir.AxisListType.X,
                                    op=mybir.AluOpType.add)
            nc.vector.tensor_scalar_add(a1_all[:], a1_all[:], le1b[:, 0:1])
            nc.vector.tensor_mul(scrT[:], xp_all[:],
                                 le2[:].unsqueeze(1).to_broadcast([P, T, D]))
            nc.vector.tensor_reduce(b2_all[:], scrT[:],
                                    axis=mybir.AxisListType.X,
                                    op=mybir.AluOpType.add)
            nc.vector.tensor_mul(scrT[:], xp_all[:],
                                 le3[:].unsqueeze(1).to_broadcast([P, T, D]))
            nc.vector.tensor_reduce(l3_all[:], scrT[:],
                                    axis=mybir.AxisListType.X,
                                    op=mybir.AluOpType.add)
            nc.sync.dma_start(xp_o[:], xp_all[:])
            nc.sync.dma_start(a1_o[:], a1_all[:])
            nc.sync.dma_start(b2_o[:], b2_all[:])
            nc.sync.dma_start(l3_o[:], l3_all[:])

    nc.compile()
    return nc


# ----------------------------------------------------------------------------
# Launch 2: LEConv aggregate + fitness + output
# ----------------------------------------------------------------------------

def _build_l2(K_t, offs, SF):
    nc = bacc.Bacc("TRN2", target_bir_lowering=False)
    a1e_d = nc.dram_tensor("a1exp", [P, SF], F32, kind="ExternalInput")
    xp_d = nc.dram_tensor("xp_in", [P, T, D], F32, kind="ExternalInput")
    b2_d = nc.dram_tensor("b2_in", [P, T], F32, kind="ExternalInput")
    l3_d = nc.dram_tensor("l3_in", [P, T], F32, kind="ExternalInput")
    deg_d = nc.dram_tensor("deg", [P, T], F32, kind="ExternalInput")
    l3b_d = nc.dram_tensor("le3b", [P, 1], F32, kind="ExternalInput")

    out0_o = nc.dram_tensor("out0", [P, T, D], F32, kind="ExternalOutput")
    fit_o = nc.dram_tensor("fit_out", [P, T], F32, kind="ExternalOutput")

    with tile.TileContext(nc) as tc:
        with tc.tile_pool(name="single", bufs=1) as single, \
             tc.tile_pool(name="pool", bufs=4) as pool:
            a1e_sb = single.tile([P, SF], F32)
            xp_sb = single.tile([P, T, D], F32)
            b2_sb = single.tile([P, T], F32)
            l3_sb = single.tile([P, T], F32)
            deg_sb = single.tile([P, T], F32)
            l3b = single.tile([P, 1], F32)
            out0_all = single.tile([P, T, D], F32)
            fit_all = single.tile([P, T], F32)
            agg_all = single.tile([P, T], F32)

            nc.sync.dma_start(a1e_sb[:], a1e_d[:])
            nc.sync.dma_start(xp_sb[:], xp_d[:])
            nc.sync.dma_start(b2_sb[:], b2_d[:])
            nc.sync.dma_start(l3_sb[:], l3_d[:])
            nc.sync.dma_start(deg_sb[:], deg_d[:])
            nc.sync.dma_start(l3b[:], l3b_d[:])

            for t in range(T):
                K = K_t[t]
                off = int(offs[t])
                nc.vector.tensor_reduce(agg_all[:, t:t + 1],
                                        a1e_sb[:, off:off + K],
                                        axis=mybir.AxisListType.X,
                                        op=mybir.AluOpType.add)
            # pre = agg - deg*b2 + l3 ; fit = sigmoid(pre + le3_b)
            nb = single.tile([P, T], F32)
            nc.vector.tensor_mul(nb[:], deg_sb[:], b2_sb[:])
            pre = single.tile([P, T], F32)
            nc.vector.tensor_sub(pre[:], agg_all[:], nb[:])
            nc.vector.tensor_add(pre[:], pre[:], l3_sb[:])
            nc.scalar.activation(fit_all[:], pre[:],
                                 mybir.ActivationFunctionType.Sigmoid,
                                 bias=l3b[:, 0:1], scale=1.0)
            for t in range(T):
                nc.vector.tensor_scalar_mul(out0_all[:, t, :], xp_sb[:, t, :],
                                            fit_all[:, t:t + 1])

            nc.sync.dma_start(out0_o[:], out0_all[:])
            nc.sync.dma_start(fit_o[:], fit_all[:])

    nc.compile()
    return nc


# ----------------------------------------------------------------------------
# Entry point
# ----------------------------------------------------------------------------

def kernel(x, edge_index, lin_w, lin_b, att_w, att_b,
           le1_w, le1_b, le2_w, le3_w, le3_b):
    x = np.ascontiguousarray(np.asarray(x, dtype=np.float32))
    pp = _prep(np.asarray(edge_index))
    key = (pp["SF"], tuple(pp["K_lo"]), tuple(pp["K_hi"]))
    if key not in _compiled:
        _compiled[key] = (
            _build_l1(pp["K_lo"], pp["K_hi"], pp["offs"], pp["lo_offs"],
                      pp["hi_offs"], pp["SF"]),
            _build_l2(pp["K_t"], pp["offs"], pp["SF"]))
    nc1, nc2 = _compiled[key]

    pad = np.full((1, D), -1e30, np.float32)
    xext = np.concatenate([pad, x, pad], axis=0)
    att_lo = np.asarray(att_w[0, :D], np.float32)
    att_hi = np.asarray(att_w[0, D:], np.float32)
    cvec = np.asarray(lin_w, np.float32).T @ att_lo
    u0 = np.float32(att_lo @ np.asarray(lin_b, np.float32) + att_b[0])

    def rep(vec):
        return np.broadcast_to(np.asarray(vec, np.float32).reshape(1, -1),
                               (P, int(np.size(vec)))).copy()

    in1 = []
    for c in range(NCORES):
        in1.append(dict(
            xext=xext, idxlo=pp["idx_lo"][c], idxhi=pp["idx_hi"][c],
            mask=pp["mask"][c],
            atthi=rep(att_hi), cvec=rep(cvec), u0=rep([u0]),
            le1=rep(le1_w[0]), le2=rep(le2_w[0]), le3=rep(le3_w[0]),
            le1b=rep([le1_b[0]]),
        ))
    res1 = run_bass_kernel_spmd(nc1, in1, core_ids=list(range(NCORES)),
                                trace=TRACE)
    r1 = res1.results
    LAST_TIMES.clear()
    LAST_TIMES.append(res1.exec_time_ns)

    a1_full = np.concatenate([r1[c]["a1_out"].ravel() for c in range(NCORES)])
    a1exp = a1_full[pp["idx2"]].astype(np.float32)      # [NCORES, P, SF]
    a1exp *= pp["mask"]

    in2 = []
    for c in range(NCORES):
        in2.append(dict(
            a1exp=a1exp[c],
            xp_in=r1[c]["xp_out"], b2_in=r1[c]["b2_out"],
            l3_in=r1[c]["l3_out"], deg=pp["deg_grid"][c],
            le3b=rep([le3_b[0]]),
        ))
    res2 = run_bass_kernel_spmd(nc2, in2, core_ids=list(range(NCORES)),
                                trace=TRACE)
    r2 = res2.results
    LAST_TIMES.append(res2.exec_time_ns)

    out0 = np.empty((N, D), np.float32)
    fit = np.empty((N,), np.float32)
    ip = np.arange(STRIP)
    pj, tj = ip % P, ip // P
    for c in range(NCORES):
        out0[pp["perm"][c]] = r2[c]["out0"][pj, tj]
        fit[pp["perm"][c]] = r2[c]["fit_out"][pj, tj]
    return out0, fit
